# revision 4
# baseline (speedup 1.0000x reference)
"""MetaFeatureExtractor Trainium2 kernel.

Computes per-sample statistics over the time axis of x [B, T, C]:
  out = concat([mean, std(ddof=1), max, min, slope], axis=1) -> [B, 5C]

Sharding: pure data parallel over 8 NeuronCores (B=256 -> 32 samples/core).

Per-core layout: x_shard [32, 2048, 64] is loaded in 8 tiles of 4 samples:
  SBUF tile [128 partitions, (s=4, j=16, c=64)] where partition p holds
  T-rows [16p, 16p+16) of each sample -> 4 KiB contiguous DMA runs.

Design (v2, bf16-centric): the tolerance gate (rel_err < 2e-2) admits bf16
(~2e-3), which unlocks the DVE 2x packed-16-bit perf mode and full-rate PE
matmuls, so one ACT cast pass feeds every other engine:
  ACT    : f32 -> bf16 cast of each tile (the only full pass on ACT),
           PSUM extraction copies, sqrt for std
  DVE    : max / min over j via contiguous-block bf16 tensor_tensor trees
           (2x mode: packed 2-byte SBUF operands)
  PE     : sum(x) via ones-weight bf16 matmuls; sum(x^2) via 2-sample
           Gram matmuls (xb^T @ xb, diag extracted downstream)
  GPSIMD : negation for min-via-max, per-tile partition_all_reduce folds
           (max/min/Q-diag-sum), Gram diag masking
Max/min are exact at bf16 resolution (rounding is monotonic); sums keep
fp32 PSUM accumulation over bf16 inputs (overall rel err ~1e-3).
"""

import threading

import numpy as np

B_TOTAL = 256
N_CORES = 8
B = B_TOTAL // N_CORES  # 32 samples per core
T = 2048
C = 64
S_PER_TILE = 4
N_TILES = B // S_PER_TILE  # 8
J = 16                      # T-rows per partition per tile
P = 128                     # partitions
G = S_PER_TILE // 2         # 2-sample Gram blocks per tile
OUT_COLS = 5 * C            # 320

_cache = threading.local()


def _build(
    do_endpoint=True,
    do_reduce=True,
    do_mm=True,
    do_par=True,
    do_scatter=True,
    n_tiles=N_TILES,
    rep=1,
    loop_n=0,
    mask_on_dve=False,
):
    import concourse.bacc as bacc
    import concourse.bass as bass
    import concourse.tile as tile
    from concourse import bass_isa, mybir

    f32 = mybir.dt.float32
    bf16 = mybir.dt.bfloat16
    AF = mybir.ActivationFunctionType
    Alu = mybir.AluOpType

    nc = bacc.Bacc("TRN2", target_bir_lowering=False, debug=False)

    x_ap = nc.dram_tensor("x", [B, T, C], f32, kind="ExternalInput").ap()
    # diag mask for Gram extraction: mk[m, g, n] = (m == n)
    mk_ap = nc.dram_tensor("mask", [P, G, P], f32, kind="ExternalInput").ap()
    y_ap = nc.dram_tensor("y", [B, OUT_COLS], f32, kind="ExternalOutput").ap()

    import contextlib

    with tile.TileContext(nc) as tc:
      for _rep in range(rep):
        loop_cm = tc.For_i(0, loop_n, 1) if loop_n else contextlib.nullcontext()
        with (
            loop_cm,
            tc.tile_pool(name="xin", bufs=3) as xpool,
            tc.tile_pool(name="xbf", bufs=3) as xbpool,
            tc.tile_pool(name="tree", bufs=2) as tree_pool,
            tc.tile_pool(name="gram", bufs=2) as gram_pool,
            tc.tile_pool(name="persist", bufs=1) as pers,
            tc.tile_pool(name="small", bufs=1) as small,
            tc.tile_pool(name="ps", bufs=4, space="PSUM") as pspool,
        ):
            # persistent accumulators / partials
            Mxb = pers.tile([P, N_TILES, S_PER_TILE, C], bf16, tag="Mxb")
            Mnb = pers.tile([P, N_TILES, S_PER_TILE, C], bf16, tag="Mnb")
            ARmax = pers.tile([P, N_TILES * S_PER_TILE * C], bf16, tag="ARmax")
            ARmin = pers.tile([P, N_TILES * S_PER_TILE * C], bf16, tag="ARmin")
            ARQ = pers.tile([P, N_TILES * S_PER_TILE * C], f32, tag="ARQ")
            SROW = pers.tile([1, B * C], f32, tag="SROW")
            if not do_mm or n_tiles < N_TILES:
                nc.vector.memset(SROW[:], 0.0)
                nc.vector.memset(ARQ[:], 0.0)
            if not do_reduce or not do_par or n_tiles < N_TILES:
                nc.vector.memset(ARmax[:], 0.0)
                nc.vector.memset(ARmin[:], 0.0)

            ones_b = small.tile([P, 1], bf16, tag="ones_b")
            nc.vector.memset(ones_b[:], 1.0)
            M2 = small.tile([P, G, P], f32, tag="M2")
            nc.scalar.dma_start(out=M2[:], in_=mk_ap[:])
            # warm the sqrt table set so the tail std-sqrt pays no table load
            ones_f = small.tile([1, 1], f32, tag="ones_f")
            nc.vector.memset(ones_f[:], 1.0)
            sqrt_warm = small.tile([1, 1], f32, tag="sqrt_warm")
            nc.scalar.activation(sqrt_warm[:], ones_f[:], AF.Sqrt)

            OUT = small.tile([B, OUT_COLS], f32, tag="OUT")
            E = small.tile([B, 2, C], f32, tag="endpoints")
            S32 = small.tile([B, C], f32, tag="S32")
            Q32 = small.tile([B, C], f32, tag="Q32")
            MXb = small.tile([B, C], bf16, tag="MXb")
            MNb = small.tile([B, C], bf16, tag="MNb")
            TMP1 = small.tile([B, C], f32, tag="TMP1")
            TMP2 = small.tile([B, C], f32, tag="TMP2")

            # endpoint rows for slope: x[:, 0, :] and x[:, T-1, :]
            if do_endpoint:
                nc.scalar.dma_start(out=E[:], in_=x_ap[:, 0 : T : T - 1, :])
            else:
                nc.vector.memset(E[:], 0.0)

            for i in range(n_tiles):
                xt = xpool.tile([P, S_PER_TILE, J, C], f32, tag="xt")
                src = x_ap[i * S_PER_TILE : (i + 1) * S_PER_TILE].rearrange(
                    "s (p j) c -> p s j c", p=P, j=J
                )
                nc.sync.dma_start(out=xt[:], in_=src)

                # ACT: the single full-rate pass -> bf16 working copy
                xb = xbpool.tile([P, S_PER_TILE, J, C], bf16, tag="xb")
                nc.scalar.copy(xb[:], xt[:])

                # DVE: max / min over j via contiguous-block bf16 TT trees
                if do_reduce:
                    for op, dst in ((Alu.max, Mxb), (Alu.min, Mnb)):
                        tA = tree_pool.tile([P, S_PER_TILE, J // 2, C], bf16, tag="tA")
                        nc.vector.tensor_tensor(
                            out=tA[:], in0=xb[:, :, 0 : J // 2, :],
                            in1=xb[:, :, J // 2 :, :], op=op,
                        )
                        tB = tree_pool.tile([P, S_PER_TILE, J // 4, C], bf16, tag="tB")
                        nc.vector.tensor_tensor(
                            out=tB[:], in0=tA[:, :, 0 : J // 4, :],
                            in1=tA[:, :, J // 4 :, :], op=op,
                        )
                        tC = tree_pool.tile([P, S_PER_TILE, J // 8, C], bf16, tag="tC")
                        nc.vector.tensor_tensor(
                            out=tC[:], in0=tB[:, :, 0 : J // 8, :],
                            in1=tB[:, :, J // 8 :, :], op=op,
                        )
                        nc.vector.tensor_tensor(
                            out=dst[:, i, :, :], in0=tC[:, :, 0, :],
                            in1=tC[:, :, 1, :], op=op,
                        )
                    if do_par:
                        NegMnb = tree_pool.tile([P, S_PER_TILE, C], bf16, tag="NegMnb")
                        nc.gpsimd.tensor_scalar_mul(NegMnb[:], Mnb[:, i, :, :], -1.0)
                        nc.gpsimd.partition_all_reduce(
                            out_ap=ARmax[:, bass.ts(i, S_PER_TILE * C)],
                            in_ap=Mxb[:, i, :, :].rearrange("p s c -> p (s c)"),
                            channels=P,
                            reduce_op=bass_isa.ReduceOp.max,
                        )
                        nc.gpsimd.partition_all_reduce(
                            out_ap=ARmin[:, bass.ts(i, S_PER_TILE * C)],
                            in_ap=NegMnb[:].rearrange("p s c -> p (s c)"),
                            channels=P,
                            reduce_op=bass_isa.ReduceOp.max,
                        )

                if do_mm:
                    # PE: sum(x) via ones-weight bf16 matmuls over j
                    psS = pspool.tile([1, S_PER_TILE * C], f32, tag="psS")
                    for j in range(J):
                        nc.tensor.matmul(
                            out=psS[:],
                            lhsT=ones_b[:],
                            rhs=xb[:, :, j, :],
                            start=(j == 0),
                            stop=(j == J - 1),
                        )
                    # PE: sum(x^2) via 2-sample Gram blocks xb^T @ xb
                    pst = pspool.tile([P, G, P], f32, tag="pst")
                    for g in range(G):
                        blk = xb[:, 2 * g : 2 * g + 2, :, :]
                        for j in range(J):
                            nc.tensor.matmul(
                                out=pst[:, g, :],
                                lhsT=blk[:, :, j, :],
                                rhs=blk[:, :, j, :],
                                start=(j == 0),
                                stop=(j == J - 1),
                            )
                    nc.scalar.copy(SROW[0:1, bass.ts(i, S_PER_TILE * C)], psS[:])

                    # Gram diag -> Q row: mask then fold partitions (sum)
                    if mask_on_dve:
                        msk = gram_pool.tile([P, G, P], f32, tag="msk")
                        nc.vector.tensor_tensor(
                            out=msk[:], in0=pst[:], in1=M2[:], op=Alu.mult
                        )
                    else:
                        gsb = gram_pool.tile([P, G, P], f32, tag="gsb")
                        nc.scalar.copy(gsb[:], pst[:])
                        msk = gram_pool.tile([P, G, P], f32, tag="msk")
                        nc.gpsimd.tensor_tensor(
                            out=msk[:], in0=gsb[:], in1=M2[:], op=Alu.mult
                        )
                    nc.gpsimd.partition_all_reduce(
                        out_ap=ARQ[:, bass.ts(i, S_PER_TILE * C)],
                        in_ap=msk[:].rearrange("p g n -> p (g n)"),
                        channels=P,
                        reduce_op=bass_isa.ReduceOp.add,
                    )

            # scatter rows [1, B*C] -> [B, C] tiles / output columns
            if do_scatter:
                nc.scalar.dma_start(out=MXb[:], in_=ARmax[0:1, :])
                nc.scalar.dma_start(out=MNb[:], in_=ARmin[0:1, :])
                nc.scalar.dma_start(out=S32[:], in_=SROW[0:1, :])
                nc.scalar.dma_start(out=Q32[:], in_=ARQ[0:1, :])
            else:
                nc.vector.memset(MXb[:], 0.0)
                nc.vector.memset(MNb[:], 0.0)
                nc.vector.memset(S32[:], 0.0)
                nc.vector.memset(Q32[:], 0.0)

            # max (bf16 -> f32); min = -(max of negated)
            nc.scalar.copy(OUT[:, 2 * C : 3 * C], MXb[:])
            nc.scalar.mul(OUT[:, 3 * C : 4 * C], MNb[:], -1.0)

            # mean = S / T
            nc.vector.tensor_scalar_mul(OUT[:, 0:C], S32[:], 1.0 / T)
            # var = (Q - S * mean) / (T - 1); std = sqrt(var)
            nc.vector.tensor_tensor(
                out=TMP1[:], in0=S32[:], in1=OUT[:, 0:C], op=Alu.mult
            )
            nc.vector.tensor_sub(TMP2[:], Q32[:], TMP1[:])
            nc.vector.tensor_scalar_mul(TMP2[:], TMP2[:], 1.0 / (T - 1))
            nc.scalar.activation(OUT[:, C : 2 * C], TMP2[:], AF.Sqrt)

            # slope = (x[:, -1, :] - x[:, 0, :]) / (T - 1)
            nc.vector.tensor_sub(TMP1[:], E[:, 1, :], E[:, 0, :])
            nc.vector.tensor_scalar_mul(OUT[:, 4 * C : 5 * C], TMP1[:], 1.0 / (T - 1))

            nc.sync.dma_start(out=y_ap, in_=OUT[:])

    nc.compile()
    return nc


def _mask_np():
    mk = np.zeros((P, G, P), dtype=np.float32)
    for m in range(P):
        mk[m, :, m] = 1.0
    return mk


def _get_nc():
    if getattr(_cache, "nc", None) is None:
        _cache.nc = _build()
    return _cache.nc


def _in_maps(x):
    mk = _mask_np()
    return [{"x": x[k * B : (k + 1) * B], "mask": mk} for k in range(N_CORES)]


def kernel(x: np.ndarray) -> np.ndarray:
    from concourse.bass_utils import run_bass_kernel_spmd

    x = np.ascontiguousarray(x, dtype=np.float32)
    assert x.shape == (B_TOTAL, T, C), x.shape

    nc = _get_nc()
    in_maps = _in_maps(x)
    last_err = None
    for _attempt in range(3):
        try:
            res = run_bass_kernel_spmd(nc, in_maps, list(range(N_CORES)))
            break
        except Exception as e:  # transient axon transfer errors — retry
            last_err = e
    else:
        raise last_err
    return np.concatenate([res.results[k]["y"] for k in range(N_CORES)], axis=0)


def _build_repeat(rep):
    return _build(rep=rep)


def _build_loop(n):
    return _build(loop_n=n)
